# revision 16
# baseline (speedup 1.0000x reference)
"""DecoderRNN (show-attend-tell) Trainium2 kernel.

Sharding: batch-parallel recurrence (4 batches/core on 8 cores), one
AllGather of hidden states, vocab-sharded output projection (4000
vocab rows/core, W_out shard SBUF-resident).

V2: all matmul operands bf16 (4x PE throughput vs fp32, half the DMA),
batched softmax (one exp/recip/mult over all 4 local batches),
attention bias pre-folded into enc_att, single-copy h handoff.

Layouts (per core, local batches b in 0..3):
  folded gate layout: ps_g[32j+b, m] = gates[b, 512*(m//128) + 128*j + m%128]
  hallT[kk, t, 32j+b] = h_t[b, 128j+kk]   (via PE transpose of h_fold)

Constants are packed into 3 combined tensors (cst0/cstA/woC) so each
phase's matmuls wait on a single DMA semaphore.
"""
import os
import numpy as np
import ml_dtypes

B, P, E, H, A, V, T = 32, 196, 512, 512, 512, 32000, 50
R = 8
BL = B // R     # 4
VL = V // R     # 4000
NKT = 12        # gates K-tiles: 4 xe + 4 awe + 4 h
BF = ml_dtypes.bfloat16


def _layoutA(BT):
    off, d = 0, {}
    for name, sz in [("fa", BL * E), ("fb", BL * E), ("xeT", 4 * BT),
                     ("wde", 4 * A), ("wxp", NKT * 4 * 512), ("wf", 4),
                     ("bg", 4 * 512), ("idn", 128),
                     ("ons", 128), ("z4", 4)]:
        d[name] = (off, sz)
        off += sz
    return d, off

_cache = {}


def _build(nT: int):
    import concourse.bass as bass
    import concourse.bacc as bacc
    import concourse.mybir as mybir
    from concourse import tile

    f32 = mybir.dt.float32
    bf16 = mybir.dt.bfloat16
    nc = bacc.Bacc(None, target_bir_lowering=False)
    BT = nT * BL
    LA, NA = _layoutA(BT)
    N0 = 4 * BL * P + 4 * A          # cst0: ft + wen
    NW = 4 * VL                      # woC: wo

    cst0_d = nc.dram_tensor("cst0", [128, N0], bf16, kind="ExternalInput")
    cstA_d = nc.dram_tensor("cstA", [128, NA], bf16, kind="ExternalInput")
    cstF_d = nc.dram_tensor("cstF", [128, 4], f32, kind="ExternalInput")
    woC_d = nc.dram_tensor("woC", [128, NW], bf16, kind="ExternalInput")
    out_d = nc.dram_tensor("out", [R * BT, VL], bf16, kind="ExternalOutput")
    t0a = nT // 2
    t0b = max(t0a, nT - max(1, nT // 8))
    M1 = 4 * t0a
    M2 = 4 * (t0b - t0a)
    M3 = BT - M1 - M2
    hloc1 = nc.dram_tensor("hloc1", [H, M1], bf16)
    hag1 = nc.dram_tensor("hag1", [R * H, M1], bf16, addr_space="Shared")
    if M2:
        hloc2 = nc.dram_tensor("hloc2", [H, M2], bf16)
        hag2 = nc.dram_tensor("hag2", [R * H, M2], bf16, addr_space="Shared")
    hloc3 = nc.dram_tensor("hloc3", [H, M3], bf16)
    hag3 = nc.dram_tensor("hag3", [R * H, M3], bf16, addr_space="Shared")

    Relu = mybir.ActivationFunctionType.Relu
    Sig = mybir.ActivationFunctionType.Sigmoid
    Tanh = mybir.ActivationFunctionType.Tanh
    Exp = mybir.ActivationFunctionType.Exp
    add_op = mybir.AluOpType.add
    mult_op = mybir.AluOpType.mult
    max_op = mybir.AluOpType.max

    with tile.TileContext(nc) as tc:
        import contextlib
        with contextlib.ExitStack() as es:
            statep = es.enter_context(tc.tile_pool(name="state", bufs=1))
            # enc_att with (b_enc+b_dec) bias pre-added: [kk,(q,b,p)]
            encB = statep.tile([128, 4, BL, P], bf16)
            hallT = statep.tile([128, nT, 128], bf16)  # [kk, t, 32j+b]
            # FW[p, pt, b, j, m] = (features[b] @ W_awe.T)[p + 128*pt,
            #                       512*(m//128) + 128*j + m%128]
            FW = statep.tile([128, 2, BL, 4, 512], bf16)
            woC = statep.tile([128, NW], bf16)
            nc.sync.dma_start(woC[:], woC_d[:])
            wo = woC.rearrange("p (k v) -> p k v", k=4)
            # per-batch alpha columns (zero except col b) for the gates
            # matmuls: aTz1[:, b, :] is [128p, 4] with alpha_b at col b
            aTz1 = statep.tile([128, BL, 4], bf16)
            aTz2 = statep.tile([P - 128, BL, 4], bf16)
            nc.vector.memset(aTz1[:], 0.0)
            nc.vector.memset(aTz2[:], 0.0)

            cA_cm = tc.tile_pool(name="cA", bufs=1)
            cA = cA_cm.__enter__()
            cstA = cA.tile([128, NA], bf16)
            nc.sync.dma_start(cstA[:], cstA_d[:])
            battF = cA.tile([128, 4], f32)
            nc.sync.dma_start(battF[:], cstF_d[:])

            def sA(name):
                o, sz = LA[name]
                return cstA[:, o:o + sz]
            fa = sA("fa").rearrange("p (b e) -> p b e", b=BL)
            fb_ = sA("fb").rearrange("p (b e) -> p b e", b=BL)
            xeT = sA("xeT").rearrange("p (k t) -> p k t", k=4)
            wde = sA("wde").rearrange("p (k a) -> p k a", k=4)
            wxp = sA("wxp").rearrange("p (k j m) -> p k j m", k=NKT, j=4)
            wf = sA("wf")
            bg = sA("bg").rearrange("p (j m) -> p j m", j=4)
            idn = sA("idn")
            ons = sA("ons")
            z4 = sA("z4")

            # ---- phase 0: enc_attT (+ att bias) -----------------
            with tc.tile_pool(name="c0", bufs=1) as c0, \
                 tc.tile_pool(name="ps0", bufs=2,
                              space=bass.MemorySpace.PSUM) as ps0:
                cst0 = c0.tile([128, N0], bf16)
                nc.sync.dma_start(cst0[:], cst0_d[:])
                ft = cst0[:, 0:4 * BL * P].rearrange(
                    "p (k b q) -> p k b q", k=4, b=BL)
                wen = cst0[:, 4 * BL * P:].rearrange("p (k a) -> p k a", k=4)
                for q in range(4):
                    for b in range(BL):
                        ep = ps0.tile([128, P], f32, tag="ep")
                        for kt in range(4):
                            nc.tensor.matmul(
                                ep[:], wen[:, kt, 128 * q:128 * (q + 1)],
                                ft[:, kt, b, :],
                                start=(kt == 0), stop=(kt == 3))
                        nc.vector.tensor_scalar(
                            encB[:, q, b, :], ep[:], battF[:, q:q + 1],
                            None, add_op)
                # FW = features @ W_awe.T (wxp kt 4..7 holds W_awe fold)
                for b in range(BL):
                    for pt in range(2):
                        PL = 128 if pt == 0 else P - 128
                        for j in range(4):
                            psF = ps0.tile([128, 512], f32, tag="fw",
                                           bufs=2)
                            for kt in range(4):
                                nc.tensor.matmul(
                                    psF[0:PL, :],
                                    ft[:, kt, b, 128 * pt:128 * pt + PL],
                                    wxp[:, 4 + kt, j, :],
                                    start=(kt == 0), stop=(kt == 3))
                            nc.vector.tensor_copy(
                                FW[0:PL, pt, b, j, :], psF[0:PL, :])

            # ---- phase 1: recurrence ----------------------------
            c_prev = statep.tile([128, 128], f32, tag="cst0")
            nc.vector.memset(c_prev[:], 0.0)

            p1_cm = tc.tile_pool(name="p1", bufs=2)
            p1ps_cm = tc.tile_pool(name="p1ps", bufs=1,
                                   space=bass.MemorySpace.PSUM)
            p1 = p1_cm.__enter__()
            p1ps = p1ps_cm.__enter__()
            p3i_cm = tc.tile_pool(name="p3i", bufs=2)
            p3i = p3i_cm.__enter__()

            NCH = VL // 500
            units = [(r, nch) for r in range(R) for nch in range(NCH)]
            ucur = [0]
            hg1_tiles = {}

            def emit_unit():
                if ucur[0] >= len(units):
                    return
                r, nch = units[ucur[0]]
                ucur[0] += 1
                if r not in hg1_tiles:
                    hg = p3i.tile([128, 4, M1], bf16, tag="hg")
                    nc.sync.dma_start(
                        hg[:], hag1[H * r:H * (r + 1), :].rearrange(
                            "(k kk) t -> kk k t", kk=128))
                    hg1_tiles[r] = hg
                hg = hg1_tiles[r]
                pt = p1ps.tile([128, 500], f32, tag="p3")
                for kt in range(4):
                    nc.tensor.matmul(
                        pt[0:M1, :], hg[:, kt, :],
                        wo[:, kt, 500 * nch:500 * (nch + 1)],
                        start=(kt == 0), stop=(kt == 3))
                ostrip = p3i.tile([128, 500], bf16, tag="os")
                nc.vector.tensor_copy(ostrip[0:M1, :], pt[0:M1, :])
                nc.sync.dma_start(
                    out_d[BT * r:BT * r + M1,
                          500 * nch:500 * (nch + 1)],
                    ostrip[0:M1, :])

            # gates psum for step 0: bias + xe emitted up front
            ps_g = p1ps.tile([128, 512], f32, tag="g", bufs=2)
            for j in range(4):
                nc.tensor.matmul(
                    ps_g[32 * j:32 * j + 4, :], ons[0:1, 0:4],
                    bg[0:1, j, :], start=True, stop=False,
                    tile_position=(0, 32 * j))
            for kt in range(4):
                for j in range(4):
                    nc.tensor.matmul(
                        ps_g[32 * j:32 * j + 4, :], xeT[:, kt, 0:4],
                        wxp[:, kt, j, :], start=False, stop=False,
                        tile_position=(0, 32 * j))

            for t in range(nT):
                hT = (lambda j: hallT[:, t - 1, 32 * j:32 * j + 4]) \
                    if t > 0 else (lambda j: z4)
                # per-step budget of interleaved vocab-projection units
                # (fills PE-idle softmax/LSTM windows, keeps HAM warm)
                if t >= t0a + 2:
                    nsteps = nT - (t0a + 2)
                    i0 = (t - t0a - 2) * len(units) // nsteps
                    i1 = (t - t0a - 1) * len(units) // nsteps
                    ubudget = i1 - i0
                else:
                    ubudget = 0

                # dec_att = h @ W_dec.T, folded [32j+b, m]
                ps_dec = p1ps.tile([128, 128], f32, tag="t1")
                for kt in range(4):
                    for j in range(4):
                        nc.tensor.matmul(
                            ps_dec[32 * j:32 * j + 4, :], hT(kt),
                            wde[:, kt, 128 * j:128 * (j + 1)],
                            start=(kt == 0), stop=(kt == 3),
                            tile_position=(0, 32 * j))
                dec_sb = p1.tile([128, 128], bf16, tag="dsb")
                nc.vector.tensor_copy(dec_sb[:], ps_dec[:])
                dT_ps = p1ps.tile([128, 128], bf16, tag="t1")
                nc.tensor.transpose(dT_ps[:], dec_sb[:], idn)
                dTs = p1.tile([128, 128], f32, tag="dTs")
                nc.vector.tensor_copy(dTs[:], dT_ps[:])

                # h-part of gates: ready as soon as hallT[t-1] lands
                for kt in range(4):
                    for j in range(4):
                        nc.tensor.matmul(
                            ps_g[32 * j:32 * j + 4, :], hT(kt),
                            wxp[:, 8 + kt, j, :], start=False, stop=False,
                            tile_position=(0, 32 * j))

                # att = relu(encB + dec), bf16 [kk,(q,b,p)]
                att = p1.tile([128, 4, BL, P], bf16, tag="att")
                for q in range(4):
                    for b in range(BL):
                        bias = dTs[:, 32 * q + b:32 * q + b + 1]
                        if (q + b) % 2 == 0:
                            nc.scalar.activation(
                                att[:, q, b, :], encB[:, q, b, :], Relu,
                                bias=bias)
                        else:
                            nc.vector.tensor_scalar(
                                att[:, q, b, :], encB[:, q, b, :],
                                bias, 0.0, add_op, max_op)

                # scores: row 32b of ps_sc
                ps_sc = p1ps.tile([128, P], f32, tag="t2")
                if t == 0:
                    # unused rows feed the batched exp below: ensure
                    # the initial PSUM contents can't be inf/nan
                    nc.vector.memset(ps_sc[:], 0.0)
                for q in range(4):
                    for b in range(BL):
                        nc.tensor.matmul(
                            ps_sc[32 * b:32 * b + 1, :], wf[:, q:q + 1],
                            att[:, q, b, :],
                            start=(q == 0), stop=(q == 3),
                            tile_position=(0, 32 * b))

                for _ in range(min(2, ubudget)):
                    emit_unit()
                    ubudget -= 1

                # batched softmax over free dim (valid rows 32b)
                ex = p1.tile([128, P], f32, tag="ex")
                ssum = p1.tile([128, 1], f32, tag="ssum")
                rsum = p1.tile([128, 1], f32, tag="rsum")
                alphab = p1.tile([128, P], bf16, tag="alpha")
                nc.scalar.activation(ex[:], ps_sc[:], Exp,
                                     accum_out=ssum[:, 0:1])
                nc.vector.reciprocal(rsum[:, 0:1], ssum[:, 0:1])
                nc.vector.tensor_scalar(
                    alphab[:], ex[:], rsum[:, 0:1], None, mult_op)

                aT1_ps = p1ps.tile([128, 128], bf16, tag="t4")
                nc.tensor.transpose(aT1_ps[:], alphab[:, 0:128], idn)
                aT2_ps = p1ps.tile([P - 128, 128], bf16, tag="t5")
                nc.tensor.transpose(aT2_ps[:], alphab[:, 128:P], idn)
                for b in range(BL):
                    nc.vector.tensor_copy(
                        aTz1[:, b, b:b + 1], aT1_ps[:, 32 * b:32 * b + 1])
                    nc.vector.tensor_copy(
                        aTz2[:, b, b:b + 1], aT2_ps[:, 32 * b:32 * b + 1])

                # awe-part of gates via FW: per-(b, ptile) accumulation
                # with zero-padded alpha columns -> rows 32j+b
                nfw = 0
                for b in range(BL):
                    for pt in range(2):
                        lh = aTz1[:, b, :] if pt == 0 \
                            else aTz2[0:P - 128, b, :]
                        nfw += 1
                        for j in range(4):
                            nc.tensor.matmul(
                                ps_g[32 * j:32 * j + 4, :], lh,
                                FW[0:(128 if pt == 0 else P - 128),
                                   pt, b, j, :],
                                start=False, stop=(nfw == 8 and j == 3),
                                tile_position=(0, 32 * j))

                # LSTM tail: tanh-half for i,f,o; sigmoid = 0.5*t+0.5
                sg = p1.tile([128, 512], f32, tag="sg")
                tg = p1.tile([128, 128], f32, tag="tg")
                nc.scalar.activation(sg[:, 0:256], ps_g[:, 0:256], Tanh,
                                     scale=0.5)
                nc.scalar.activation(tg[:], ps_g[:, 256:384], Tanh)
                nc.scalar.activation(sg[:, 384:512], ps_g[:, 384:512],
                                     Tanh, scale=0.5)
                sig = p1.tile([128, 512], f32, tag="sig")
                nc.vector.tensor_scalar(sig[:, 0:256], sg[:, 0:256],
                                        0.5, 0.5, mult_op, add_op)
                tmp = p1.tile([128, 128], f32, tag="tmp")
                nc.vector.tensor_tensor(tmp[:], sig[:, 0:128], tg[:],
                                        mult_op)
                c_new = p1.tile([128, 128], f32, tag="cn")
                nc.vector.tensor_tensor(c_new[:], sig[:, 128:256],
                                        c_prev[:], mult_op)
                nc.vector.tensor_tensor(c_new[:], c_new[:], tmp[:], add_op)
                nc.vector.tensor_scalar(sig[:, 384:512], sg[:, 384:512],
                                        0.5, 0.5, mult_op, add_op)
                thc = p1.tile([128, 128], f32, tag="thc")
                nc.scalar.activation(thc[:], c_new[:], Tanh)
                h_fold = p1.tile([128, 128], bf16, tag="hf")
                nc.vector.tensor_tensor(h_fold[:], sig[:, 384:512],
                                        thc[:], mult_op)
                c_prev = c_new

                # pipeline next step's bias + xe gates into the LSTM
                # tail window (keeps the PE HAM-warm)
                if t + 1 < nT:
                    ps_g = p1ps.tile([128, 512], f32, tag="g", bufs=2)
                    for j in range(4):
                        nc.tensor.matmul(
                            ps_g[32 * j:32 * j + 4, :], ons[0:1, 0:4],
                            bg[0:1, j, :], start=True, stop=False,
                            tile_position=(0, 32 * j))
                    for kt in range(4):
                        for j in range(4):
                            nc.tensor.matmul(
                                ps_g[32 * j:32 * j + 4, :],
                                xeT[:, kt, 4 * (t + 1):4 * (t + 1) + 4],
                                wxp[:, kt, j, :], start=False, stop=False,
                                tile_position=(0, 32 * j))

                for _ in range(ubudget):
                    emit_unit()

                hT_ps = p1ps.tile([128, 128], bf16, tag="t3")
                nc.tensor.transpose(hT_ps[:], h_fold[:], idn)
                nc.vector.tensor_copy(hallT[:, t, :], hT_ps[:])

                if t == t0a - 1:
                    # first-chunk h AllGather overlaps the rest of the
                    # recurrence
                    for j in range(4):
                        nc.sync.dma_start(
                            hloc1[128 * j:128 * (j + 1), :].rearrange(
                                "kk (t b) -> kk t b", b=BL),
                            hallT[:, 0:t0a, 32 * j:32 * j + 4])
                    nc.gpsimd.collective_compute(
                        "AllGather", mybir.AluOpType.bypass,
                        ins=[hloc1[:]], outs=[hag1[:]],
                        replica_groups=[list(range(R))])
                if M2 and t == t0b - 1 and t0b > t0a:
                    for j in range(4):
                        nc.sync.dma_start(
                            hloc2[128 * j:128 * (j + 1), :].rearrange(
                                "kk (t b) -> kk t b", b=BL),
                            hallT[:, t0a:t0b, 32 * j:32 * j + 4])
                    nc.gpsimd.collective_compute(
                        "AllGather", mybir.AluOpType.bypass,
                        ins=[hloc2[:]], outs=[hag2[:]],
                        replica_groups=[list(range(R))])

            # ---- phase 2: allgather final h chunk ---------------
            for j in range(4):
                nc.sync.dma_start(
                    hloc3[128 * j:128 * (j + 1), :].rearrange(
                        "kk (t b) -> kk t b", b=BL),
                    hallT[:, t0b:nT, 32 * j:32 * j + 4])
            nc.gpsimd.collective_compute(
                "AllGather", mybir.AluOpType.bypass,
                ins=[hloc3[:]], outs=[hag3[:]],
                replica_groups=[list(range(R))])

            # leftover chunk-1 units (small nT fallback)
            while ucur[0] < len(units):
                emit_unit()

            p3i_cm.__exit__(None, None, None)
            p1ps_cm.__exit__(None, None, None)
            p1_cm.__exit__(None, None, None)
            cA_cm.__exit__(None, None, None)

            # ---- phase 3: remaining chunks ----------------------
            chunks = []
            if M2:
                chunks.append((M1, M2, hag2))
            chunks.append((M1 + M2, M3, hag3))
            with tc.tile_pool(name="p3", bufs=2) as p3, \
                 tc.tile_pool(name="p3ps", bufs=4,
                              space=bass.MemorySpace.PSUM) as p3ps:
                for r in range(R):
                    for m0, M, hagc in chunks:
                        hg = p3.tile([128, 4, M], bf16, tag="hg")
                        nc.sync.dma_start(
                            hg[:], hagc[H * r:H * (r + 1), :].rearrange(
                                "(k kk) t -> kk k t", kk=128))
                        osb = p3.tile([128, VL], bf16, tag="osb")
                        for nch in range(NCH):
                            pt = p3ps.tile([128, 500], f32, tag="pt")
                            for kt in range(4):
                                nc.tensor.matmul(
                                    pt[0:M, :], hg[:, kt, :],
                                    wo[:, kt, 500 * nch:500 * (nch + 1)],
                                    start=(kt == 0), stop=(kt == 3))
                            nc.vector.tensor_copy(
                                osb[0:M, 500 * nch:500 * (nch + 1)],
                                pt[0:M, :])
                        nc.sync.dma_start(
                            out_d[BT * r + m0:BT * r + m0 + M, :],
                            osb[0:M, :])
    nc.compile()
    return nc


def _prep_inputs(features, captions, emb, W_ih, b_ih, W_hh, b_hh,
                 W_enc, b_enc, W_dec, b_dec, W_full, b_full, W_out, b_out,
                 nT):
    f = np.float32
    BT = nT * BL
    LA, NA = _layoutA(BT)
    gidx = np.asarray(captions)[:, :nT]

    Wcat = np.concatenate(
        [np.asarray(W_ih, f)[:, :512], np.asarray(W_ih, f)[:, 512:],
         np.asarray(W_hh, f)], axis=1)               # [2048, 1536]
    Wp = Wcat.reshape(4, 4, 128, 12, 128)            # gt jj mm kt kk
    WxTp = np.ascontiguousarray(
        Wp.transpose(4, 3, 1, 0, 2)).reshape(128, NKT * 4 * 512)
    bias_n = (np.asarray(b_ih) + np.asarray(b_hh)).astype(f)
    biasg = np.zeros((128, 2048), f)
    biasg[0] = np.ascontiguousarray(
        bias_n.reshape(4, 4, 128).transpose(1, 0, 2)).reshape(2048)
    WencT = np.ascontiguousarray(
        np.asarray(W_enc, f).T.reshape(4, 128, 512)
        .transpose(1, 0, 2)).reshape(128, 2048)
    WdecT = np.ascontiguousarray(
        np.asarray(W_dec, f).T.reshape(4, 128, 512)
        .transpose(1, 0, 2)).reshape(128, 2048)
    WfT = np.zeros((128, 4), f)
    WfT[:] = np.asarray(W_full, f)[0].reshape(4, 128).T
    bias_att = np.ascontiguousarray(
        (np.asarray(b_enc) + np.asarray(b_dec)).astype(f).reshape(4, 128).T)
    ident = np.eye(128, dtype=f)
    ons = np.zeros((128, 128), f)
    ons[0] = 1.0

    in_maps = []
    for r in range(R):
        fb = np.asarray(features[BL * r:BL * (r + 1)], dtype=f)
        featA = np.ascontiguousarray(
            fb[:, :128, :].transpose(1, 0, 2)).reshape(128, BL * E)
        featB = np.zeros((128, BL * E), f)
        featB[0:P - 128] = np.ascontiguousarray(
            fb[:, 128:, :].transpose(1, 0, 2)).reshape(P - 128, BL * E)
        featT = np.ascontiguousarray(
            fb.transpose(2, 0, 1).reshape(4, 128, BL, P)
            .transpose(1, 0, 2, 3)).reshape(128, 4 * BL * P)
        g = np.asarray(emb, dtype=f)[gidx[BL * r:BL * (r + 1)]]
        xembT = np.ascontiguousarray(
            g.transpose(2, 1, 0).reshape(4, 128, BT)
            .transpose(1, 0, 2)).reshape(128, 4 * BT)

        cstA = np.zeros((128, NA), f)

        def put(name, arr):
            o, sz = LA[name]
            cstA[:, o:o + sz] = arr
        put("fa", featA)
        put("fb", featB)
        put("xeT", xembT)
        put("wde", WdecT)
        put("wxp", WxTp)
        put("wf", WfT)
        put("bg", biasg)
        put("idn", ident)
        put("ons", ons)
        put("z4", np.zeros((128, 4), f))

        cst0 = np.concatenate([featT, WencT], axis=1)

        WoT = np.ascontiguousarray(
            np.asarray(W_out[VL * r:VL * (r + 1)], dtype=f)
            .reshape(VL, 4, 128).transpose(2, 1, 0)).reshape(128, 4 * VL)
        in_maps.append(dict(cst0=cst0.astype(BF), cstA=cstA.astype(BF),
                            cstF=bias_att, woC=WoT.astype(BF)))
    return in_maps


def kernel(features, captions, emb, W_ih, b_ih, W_hh, b_hh,
           W_enc, b_enc, W_dec, b_dec, W_full, b_full, W_out, b_out,
           _nT=None, _trace=False):
    from concourse.bass_utils import run_bass_kernel_spmd
    nT = _nT or int(os.environ.get("BASS_T", T))
    if nT not in _cache:
        _cache[nT] = _build(nT)
    nc = _cache[nT]
    in_maps = _prep_inputs(
        features, captions, emb, W_ih, b_ih, W_hh, b_hh, W_enc, b_enc,
        W_dec, b_dec, W_full, b_full, W_out, b_out, nT)
    kw = dict(trace=True) if _trace else {}
    br = run_bass_kernel_spmd(nc, in_maps, list(range(R)), **kw)
    res = br.results
    if _trace:
        kernel.last_exec_ns = br.exec_time_ns
        kernel.last_profile = br.profile_json
        it = br.instructions_and_trace
        kernel.last_trace_path = it[1] if it else None
    BT = nT * BL
    out = np.empty((B, nT, V), dtype=np.float32)
    for rc in range(R):
        o = res[rc]["out"].astype(np.float32).reshape(R, nT, BL, VL)
        for rr in range(R):
            out[BL * rr:BL * (rr + 1), :, VL * rc:VL * (rc + 1)] = \
                o[rr].transpose(1, 0, 2)
    out += np.asarray(b_out, np.float32)[None, None, :]
    return out
